# revision 9
# baseline (speedup 1.0000x reference)
"""Trainium2 Bass kernel for nn_MultiHeadSelfAttention_88725434400988.

Self-contained: accepts FULL inputs, shards batch B=256 over 8 NeuronCores
(32 per core), runs one SPMD Bass program, gathers the FULL output.

Per-core algorithm (B_CORE=32, S=8, F=32, E=64, A=64, NH=2):
  - Hs, Wq, Wk, Wv, Wres cast to fp16 on host (PE matmuls run 1 cyc/row,
    fp32 PSUM accumulation; end-to-end error vs fp32 reference ~3.6e-3
    absmax / 1.9e-3 l2-relative).
  - HAM discipline: the PE clock-gate defaults to K=4/8 (1.2 GHz) and only
    reaches 2.4 GHz after ~3.4us of sustained busy; ANY ~3.4us idle window
    re-throttles. The whole kernel is structured to keep the PE dense:
    a pre-warm burst on memset data runs during the input-DMA head (so the
    Wq stream starts at full clock), the v projection is interleaved into
    the Wk stream (no sparse phase), and the attention loop is software-
    pipelined so it never stalls on ScalarE.
  - DMA queues: sync carries the Wq chunk stream + fo outputs ONLY (tiny
    wv/wres/bias DMAs go on gpsimd after hsv — each DIRECT2D issue costs
    ~650ns of engine time and was delaying the first Wq chunk); scalar
    (Activation HWDGE) carries hsT and the qt/kt partition-shift
    SBUF->SBUF DMAs so they never block the Wk prefetch on gpsimd.
  - QK projection: lhsT = 128-col tiles of W, rhs = Hs^T; psum groups of
    2 tiles (128=(jh,a), 2, 256=(b,s)); one batched copy per (group, jh).
    qt/kt are stored SIGMA-MAJOR ([64, jh, sigma, b, nh]) so the jh=1
    partition shift is a 64-descriptor contiguous-run DMA (~16KB/part);
    it is split in two halves issued as soon as their sigma range is
    complete (tg 7 / 15), hiding under the weight-stream tail.
  - Attention is TRANSPOSE-FREE: Z^T = k_chunk.T @ q (sigma on partition),
    exp on ScalarE -> bf16 (fp32 range, no overflow at |z|<=49). The act
    table is pre-loaded with a dummy Exp during the head. Denominators
    land REPLICATED on the correct partition halves straight from the PE
    (lhsT = ones(128,64), col-group packed per nh); reciprocal_approx_fast
    gives the recip tile and the normalization multiply is fused into the
    UT psum evacuation. Iteration b emits zt+exp for b+1 BEFORE den/av of
    b, so the ScalarE exp latency is covered by PE work.
  - Residual: both output groups stacked on partition halves of ONE psum
    tile (tile_position=(64,64) for g1), per 2-batch group, emitted one
    iteration late; bias+relu fused into one VectorE tensor_scalar (keeps
    ScalarE exp-only); contiguous DMA into a (128, 4096) staging output;
    host un-permutes to (B, S, F*E) fp32.
"""
import numpy as np

B, S, F, E, A, NH = 256, 8, 32, 64, 64, 2
NCORES = 8
BC = B // NCORES            # 32 batches per core
ROWS = BC * S               # 256 projection rows
CD = F * E                  # 2048 contraction dim
ND = A * F * NH             # 4096 projection cols
KTILES = CD // 128          # 16
TTILES = ND // 128          # 32 column tiles per weight
NB = BC * NH                # 64 attention batches per core
WCHUNK = 2                  # weight tiles per DMA
GT = 2                      # projection tiles batched per psum/copy group

_NC_CACHE = None


def build_bass():
    import concourse.bacc as bacc
    import concourse.tile as tile
    from concourse import mybir

    f16 = mybir.dt.float16
    bf16 = mybir.dt.bfloat16
    f32 = mybir.dt.float32
    Exp = mybir.ActivationFunctionType.Exp
    Relu = mybir.ActivationFunctionType.Relu

    nc = bacc.Bacc("TRN2", target_bir_lowering=False, debug=False)

    # host-prepped layouts (see make_in_maps)
    hst_d = nc.dram_tensor("hst", [128, KTILES, ROWS], f16, kind="ExternalInput")
    hsv_d = nc.dram_tensor("hsv", [128, NB // 2, 128], f16, kind="ExternalInput")
    wq_d = nc.dram_tensor("wq", [128, TTILES, KTILES * 128], f16,
                          kind="ExternalInput")
    wk_d = nc.dram_tensor("wk", [128, TTILES, KTILES * 128], f16,
                          kind="ExternalInput")
    wv_d = nc.dram_tensor("wv", [E, 2 * A], f16, kind="ExternalInput")
    wres_d = nc.dram_tensor("wres", [2 * A, E], f16, kind="ExternalInput")
    bias_d = nc.dram_tensor("bias", [E, 1], f32, kind="ExternalInput")
    out_d = nc.dram_tensor("out", [128, (BC // 2) * 256], f32,
                           kind="ExternalOutput")

    with tile.TileContext(nc) as tc:
        from contextlib import ExitStack
        with ExitStack() as ctx:
            singles = ctx.enter_context(tc.tile_pool(name="singles", bufs=1))

            # ---- constants / persistent tiles ----
            ones_bf = singles.tile([128, A], bf16)
            nc.vector.memset(ones_bf, 1.0)
            warm_t = singles.tile([128, 512], f16)
            nc.vector.memset(warm_t, 0.25)
            dummy_e = singles.tile([128, 8], bf16)

            hsT = singles.tile([128, KTILES, ROWS], f16)
            nc.scalar.dma_start(hsT[:, :, :], hst_d[:])
            hsv = singles.tile([128, NB // 2, 128], f16)
            nc.gpsimd.dma_start(hsv[:, :, :], hsv_d[:])

            wv_sb = singles.tile([128, 2 * A], f16)
            nc.sync.dma_start(wv_sb[0:64, :], wv_d[:])
            nc.sync.dma_start(wv_sb[64:128, :], wv_d[:])

            wres_sb = singles.tile([128, 2, E], f16)
            for half in range(2):
                for jh in range(2):
                    nc.sync.dma_start(
                        wres_sb[half * 64:(half + 1) * 64, jh, :],
                        wres_d[jh * 64:(jh + 1) * 64, :])

            bias_sb = singles.tile([128, 1], f32)
            nc.sync.dma_start(bias_sb[0:64, :], bias_d[:])
            nc.sync.dma_start(bias_sb[64:128, :], bias_d[:])

            qt = singles.tile([64, 2, BC, NH, 128], f16)
            kt_ = singles.tile([64, 2, BC, NH, 128], f16)
            v_all = singles.tile([128, NB, 2, A], bf16)
            ut = singles.tile([128, BC, 2, 128], f16)  # (nh,a) x (b, jh, f*4+sp)


            # ---- Q/K projection + batched gathers ----
            def proj_stream(w_d, dest, dma_eng, extra=None):
                with tc.tile_pool(name="wtile", bufs=4) as w_pool, \
                     tc.tile_pool(name="stage", bufs=2) as st_pool, \
                     tc.tile_pool(name="pp", bufs=3, space="PSUM") as pp_pool:
                    stage = st_pool.tile([128, BC, NH, 128], f16,
                                         name="stage", tag="stage")
                    for tg in range(TTILES // WCHUNK):
                        wt = w_pool.tile([128, WCHUNK, KTILES, 128], f16,
                                         name="wt", tag="wt")
                        dma_eng.dma_start(
                            wt[:, :, :, :],
                            w_d[:, tg * WCHUNK:(tg + 1) * WCHUNK, :]
                            .rearrange("p t (kt c) -> p t kt c", c=128))
                        for gi in range(WCHUNK // GT):
                            pp = pp_pool.tile([128, GT, ROWS], f32)
                            for ti in range(GT):
                                for kt in range(KTILES):
                                    nc.tensor.matmul(
                                        pp[:, ti, :],
                                        lhsT=wt[:, gi * GT + ti, kt, :],
                                        rhs=hsT[:, kt, :],
                                        start=(kt == 0),
                                        stop=(kt == KTILES - 1))
                            # psum free (ti, b, nh, sp) -> iterate (bn, ti, sp)
                            src = pp.rearrange(
                                "p ti (b n sp) -> p (b n) ti sp", n=NH, sp=4)
                            t0 = tg * WCHUNK + gi * GT
                            dv = dest[:, 0, :, :, :].rearrange(
                                "p b n (f sp) -> p (b n) f sp", sp=4)
                            sv = stage[:, :, :, :].rearrange(
                                "p b n (f sp) -> p (b n) f sp", sp=4)
                            nc.vector.tensor_copy(
                                dv[:, :, t0:t0 + GT, :], src[0:64])
                            nc.vector.tensor_copy(
                                sv[64:128, :, t0:t0 + GT, :], src[64:128])
                        if extra is not None:
                            extra(tg)
                    # partition shift 64..127 -> 0..63 via SBUF->SBUF DMA on
                    # the scalar queue (64 contiguous 16KB runs; doesn't
                    # head-of-line-block the Wk prefetch on gpsimd).
                    nc.scalar.dma_start(
                        dest[:, 1, :, :, :],
                        stage[64:128, :, :, :])

            proj_stream(wq_d, qt, nc.sync)

            # ---- v projection (between the streams; inputs ready early) ----
            with tc.tile_pool(name="vps", bufs=4, space="PSUM") as vps_pool:
                for bpair in range(0, NB, 2):
                    vps = [vps_pool.tile([128, 2 * A], f32, name=f"vps{i}",
                                         tag=f"vps{i}")
                           for i in range(2)]
                    for pi in range(2):
                        nc.tensor.matmul(
                            vps[pi][:, :],
                            lhsT=hsv[pi * 64:(pi + 1) * 64, bpair // 2, :],
                            rhs=wv_sb[pi * 64:(pi + 1) * 64, :],
                            start=True, stop=True,
                            tile_position=(pi * 64, 0))
                    for pi in range(2):
                        nc.vector.tensor_copy(
                            v_all[:, bpair + pi, :, :], vps[pi][:, :])

            proj_stream(wk_d, kt_, nc.gpsimd)

            # ---- PE warm-up burst ----
            # Runs right after the Wk stream with no new dependencies, so it
            # executes DURING the kt-shift DMA tail: the PE stays busy (and
            # at full p-state) through the proj->attention transition.
            with tc.tile_pool(name="warm", bufs=1, space="PSUM") as wm_pool:
                wm = wm_pool.tile([A, ROWS], f32)
                for wi in range(24):
                    nc.tensor.matmul(
                        wm[:, :], lhsT=ones_bf[:, :], rhs=hsT[:, 0, :],
                        start=(wi == 0), stop=(wi == 23))

            # ---- attention (transpose-free, software-pipelined by 1) ----
            with tc.tile_pool(name="zps", bufs=4, space="PSUM") as z_pool, \
                 tc.tile_pool(name="dps", bufs=1, space="PSUM") as d_pool, \
                 tc.tile_pool(name="aps", bufs=1, space="PSUM") as a_pool, \
                 tc.tile_pool(name="expz", bufs=4) as e_pool, \
                 tc.tile_pool(name="reps", bufs=3) as rp_pool, \
                 tc.tile_pool(name="rps", bufs=2, space="PSUM") as r_pool, \
                 tc.tile_pool(name="fo", bufs=3) as f_pool:

                def emit_zt_exp(b):
                    ezs = []
                    for nh in range(NH):
                        zt = z_pool.tile([128, 2, 256], f32, name="zt",
                                         tag="zt")
                        for h in range(2):
                            nc.tensor.matmul(
                                zt[:, h, :],
                                lhsT=kt_[:, h, b, nh, :],
                                rhs=qt[:, :, b, nh, :],
                                start=True, stop=True)
                        ez = e_pool.tile([128, 2, 256], bf16, name="ez",
                                         tag="ez")
                        ezs.append(ez)
                        nc.scalar.activation(
                            ez[:, :, :].rearrange("p a b -> p (a b)"),
                            zt[:, :, :].rearrange("p a b -> p (a b)"), Exp)
                    return ezs

                def emit_resid(bp):
                    # residual for batch pair (2*bp, 2*bp+1): both output
                    # groups stacked on partition halves of one psum tile.
                    rp = r_pool.tile([128, 2, 128], f32, name="rp", tag="rp")
                    for g in range(2):
                        for jh in range(2):
                            nc.tensor.matmul(
                                rp[g * 64:(g + 1) * 64, :, :],
                                lhsT=wres_sb[g * 64:(g + 1) * 64, jh, :],
                                rhs=ut[g * 64:(g + 1) * 64,
                                       bp * 2:(bp + 1) * 2, jh, :],
                                start=(jh == 0), stop=(jh == 1),
                                tile_position=(g * 64, g * 64))
                    fo = f_pool.tile([128, 2, 128], f32, name="fo", tag="fo")
                    nc.scalar.activation(
                        fo[:, :, :].rearrange("p a b -> p (a b)"),
                        rp[:, :, :].rearrange("p a b -> p (a b)"),
                        Relu, bias=bias_sb[:, :])
                    nc.sync.dma_start(
                        out_d[:, bp * 256:(bp + 1) * 256],
                        fo[:, :, :].rearrange("p a b -> p (a b)"))

                ezs = emit_zt_exp(0)
                for b in range(BC):
                    cur = ezs
                    if b + 1 < BC:
                        ezs = emit_zt_exp(b + 1)
                    # denominators REPLICATED onto the right partition halves
                    # directly by the PE (ones lhsT, col-group packed per nh)
                    dpr = d_pool.tile([128, 256], f32)
                    for nh in range(NH):
                        for h in range(2):
                            nc.tensor.matmul(
                                dpr[nh * 64:(nh + 1) * 64, :],
                                lhsT=ones_bf[:, :],
                                rhs=cur[nh][:, h, :],
                                start=(h == 0), stop=(h == 1),
                                tile_position=(0, nh * 64))
                    rep = rp_pool.tile([128, 256], f32)
                    nc.vector.reciprocal_approx_fast(rep[:, :], dpr[:, :])
                    av = a_pool.tile([128, 256], f32)
                    for nh in range(NH):
                        bn = b * NH + nh
                        for kk in range(2):
                            nc.tensor.matmul(
                                av[nh * 64:(nh + 1) * 64, :],
                                lhsT=v_all[:, bn, kk, :],
                                rhs=cur[nh][:, kk, :],
                                start=(kk == 0), stop=(kk == 1),
                                tile_position=(0, nh * 64))
                    nc.vector.tensor_mul(
                        ut[:, b, :, :].rearrange("p a b -> p (a b)"),
                        av[:, :], rep[:, :])
                    # residual one pair behind, so the ut writes are done
                    if b >= 2 and b % 2 == 0:
                        emit_resid(b // 2 - 1)
                emit_resid(BC // 2 - 1)
    nc.compile()
    return nc


def _get_nc():
    global _NC_CACHE
    if _NC_CACHE is None:
        _NC_CACHE = build_bass()
    return _NC_CACHE


def _prep_weight(W):
    # (CD, ND) -> (128, TTILES, KTILES*128): [p, t, kt*128+j] = W[kt*128+p, t*128+j]
    return np.ascontiguousarray(
        W.astype(np.float16).reshape(KTILES, 128, TTILES, 128)
        .transpose(1, 2, 0, 3).reshape(128, TTILES, KTILES * 128))


def make_in_maps(Hs, Wq, Wk, Wv, Wres_w, Wres_b):
    wq16 = _prep_weight(Wq)
    wk16 = _prep_weight(Wk)
    wv16 = Wv.astype(np.float16)
    wres16 = Wres_w.astype(np.float16)
    bias = Wres_b.astype(np.float32).reshape(E, 1)
    hs16 = Hs.astype(np.float16)
    maps = []
    for c in range(NCORES):
        sh = hs16[c * BC:(c + 1) * BC]                      # (BC, S, CD)
        hs2d = sh.reshape(ROWS, CD)
        hst = np.ascontiguousarray(
            hs2d.reshape(ROWS, KTILES, 128).transpose(2, 1, 0))
        # v rows in sigma' order (f*4+sp):
        # hsv[pi*64+e, q, f*4+sp] = Hs[b, nh*4+sp, f, e]; bn = 2q+pi = b*NH+nh
        arr = sh.reshape(NB, 4, F, E).transpose(0, 2, 1, 3).reshape(NB, 128, E)
        hsv = np.ascontiguousarray(
            arr.reshape(NB // 2, 2, 128, E).transpose(1, 3, 0, 2)
            .reshape(128, NB // 2, 128))
        maps.append({
            "hst": hst, "hsv": hsv,
            "wq": wq16, "wk": wk16, "wv": wv16, "wres": wres16, "bias": bias,
        })
    return maps


def _unpack_out(o):
    # o: (128, (BC//2)*256) rows (g, e), cols (bp, b2, f, sp) -> (BC, S, F*E)
    o = o.reshape(2, 64, BC // 2, 2, F, 4)
    return np.ascontiguousarray(
        o.transpose(2, 3, 0, 5, 4, 1)).reshape(BC, S, F * E)


def kernel(Hs, Wq, Wk, Wv, Wres_w, Wres_b):
    from concourse.bass_utils import run_bass_kernel_spmd
    nc = _get_nc()
    in_maps = make_in_maps(Hs, Wq, Wk, Wv, Wres_w, Wres_b)
    res = run_bass_kernel_spmd(nc, in_maps, list(range(NCORES)))
    out = np.concatenate(
        [_unpack_out(np.asarray(res.results[c]["out"]))
         for c in range(NCORES)], axis=0)
    return out.astype(np.float32)


if __name__ == "__main__":
    nc = build_bass()
    print("built OK; instructions:",
          sum(len(bb.instructions) for fn in nc.m.functions
              for bb in fn.blocks))


# revision 10
# speedup vs baseline: 1.1394x; 1.1394x over previous
"""Trainium2 Bass kernel for nn_MultiHeadSelfAttention_88725434400988.

Self-contained: accepts FULL inputs, shards batch B=256 over 8 NeuronCores
(32 per core), runs one SPMD Bass program, gathers the FULL output.

Per-core algorithm (B_CORE=32, S=8, F=32, E=64, A=64, NH=2):
  - Hs, Wq, Wk, Wv, Wres cast to fp16 on host (PE matmuls run 1 cyc/row,
    fp32 PSUM accumulation; end-to-end error vs fp32 reference ~3.6e-3
    absmax / 1.9e-3 l2-relative).
  - HAM discipline: the PE clock-gate defaults to K=4/8 (1.2 GHz) and only
    reaches 2.4 GHz after ~3.4us of sustained busy; ANY ~3.4us idle window
    re-throttles. The whole kernel is structured to keep the PE dense:
    a pre-warm burst on memset data runs during the input-DMA head (so the
    Wq stream starts at full clock), the v projection is interleaved into
    the Wk stream (no sparse phase), and the attention loop is software-
    pipelined so it never stalls on ScalarE.
  - DMA queues: sync carries the Wq chunk stream + fo outputs ONLY (tiny
    wv/wres/bias DMAs go on gpsimd after hsv — each DIRECT2D issue costs
    ~650ns of engine time and was delaying the first Wq chunk); scalar
    (Activation HWDGE) carries hsT and the qt/kt partition-shift
    SBUF->SBUF DMAs so they never block the Wk prefetch on gpsimd.
  - QK projection: lhsT = 128-col tiles of W, rhs = Hs^T; psum groups of
    2 tiles (128=(jh,a), 2, 256=(b,s)); one batched copy per (group, jh).
    qt/kt are stored SIGMA-MAJOR ([64, jh, sigma, b, nh]) so the jh=1
    partition shift is a 64-descriptor contiguous-run DMA (~16KB/part);
    it is split in two halves issued as soon as their sigma range is
    complete (tg 7 / 15), hiding under the weight-stream tail.
  - Attention is TRANSPOSE-FREE: Z^T = k_chunk.T @ q (sigma on partition),
    exp on ScalarE -> bf16 (fp32 range, no overflow at |z|<=49). The act
    table is pre-loaded with a dummy Exp during the head. Denominators
    land REPLICATED on the correct partition halves straight from the PE
    (lhsT = ones(128,64), col-group packed per nh); reciprocal_approx_fast
    gives the recip tile and the normalization multiply is fused into the
    UT psum evacuation. Iteration b emits zt+exp for b+1 BEFORE den/av of
    b, so the ScalarE exp latency is covered by PE work.
  - Residual: both output groups stacked on partition halves of ONE psum
    tile (tile_position=(64,64) for g1), per 2-batch group, emitted one
    iteration late; bias+relu fused into one VectorE tensor_scalar (keeps
    ScalarE exp-only); contiguous DMA into a (128, 4096) staging output;
    host un-permutes to (B, S, F*E) fp32.
"""
import numpy as np

B, S, F, E, A, NH = 256, 8, 32, 64, 64, 2
NCORES = 8
BC = B // NCORES            # 32 batches per core
ROWS = BC * S               # 256 projection rows
CD = F * E                  # 2048 contraction dim
ND = A * F * NH             # 4096 projection cols
KTILES = CD // 128          # 16
TTILES = ND // 128          # 32 column tiles per weight
NB = BC * NH                # 64 attention batches per core
WCHUNK = 2                  # weight tiles per DMA
GT = 2                      # projection tiles batched per psum/copy group

_NC_CACHE = None


def build_bass():
    import concourse.bacc as bacc
    import concourse.tile as tile
    from concourse import mybir

    f16 = mybir.dt.float16
    bf16 = mybir.dt.bfloat16
    f32 = mybir.dt.float32
    Exp = mybir.ActivationFunctionType.Exp
    Add = mybir.AluOpType.add
    Max = mybir.AluOpType.max

    nc = bacc.Bacc("TRN2", target_bir_lowering=False, debug=False)

    # host-prepped layouts (see make_in_maps)
    hst_d = nc.dram_tensor("hst", [128, KTILES, ROWS], f16, kind="ExternalInput")
    hsv_d = nc.dram_tensor("hsv", [128, NB // 2, 128], f16, kind="ExternalInput")
    wq_d = nc.dram_tensor("wq", [128, TTILES, KTILES * 128], f16,
                          kind="ExternalInput")
    wk_d = nc.dram_tensor("wk", [128, TTILES, KTILES * 128], f16,
                          kind="ExternalInput")
    wv_d = nc.dram_tensor("wv", [E, 2 * A], f16, kind="ExternalInput")
    wres_d = nc.dram_tensor("wres", [2 * A, E], f16, kind="ExternalInput")
    bias_d = nc.dram_tensor("bias", [E, 1], f32, kind="ExternalInput")
    out_d = nc.dram_tensor("out", [128, (BC // 2) * 256], f32,
                           kind="ExternalOutput")

    with tile.TileContext(nc) as tc:
        from contextlib import ExitStack
        with ExitStack() as ctx:
            singles = ctx.enter_context(tc.tile_pool(name="singles", bufs=1))

            # ---- constants / persistent tiles ----
            ones_bf = singles.tile([128, A], bf16)
            nc.vector.memset(ones_bf, 1.0)
            warm_t = singles.tile([128, 512], f16)
            nc.vector.memset(warm_t, 0.25)
            dummy_e = singles.tile([128, 8], bf16)

            hsT = singles.tile([128, KTILES, ROWS], f16)
            nc.scalar.dma_start(hsT[:, :, :], hst_d[:])
            hsv = singles.tile([128, NB // 2, 128], f16)
            nc.gpsimd.dma_start(hsv[:, :, :], hsv_d[:])

            wv_sb = singles.tile([128, 2 * A], f16)
            nc.gpsimd.dma_start(wv_sb[0:64, :], wv_d[:])
            nc.gpsimd.dma_start(wv_sb[64:128, :], wv_d[:])

            wres_sb = singles.tile([128, 2, E], f16)
            for half in range(2):
                for jh in range(2):
                    nc.gpsimd.dma_start(
                        wres_sb[half * 64:(half + 1) * 64, jh, :],
                        wres_d[jh * 64:(jh + 1) * 64, :])

            bias_sb = singles.tile([128, 1], f32)
            nc.gpsimd.dma_start(bias_sb[0:64, :], bias_d[:])
            nc.gpsimd.dma_start(bias_sb[64:128, :], bias_d[:])

            qt = singles.tile([64, 2, BC, NH, 128], f16)
            kt_ = singles.tile([64, 2, BC, NH, 128], f16)
            v_all = singles.tile([128, NB, 2, A], bf16)
            ut = singles.tile([128, BC, 2, 128], f16)  # (nh,a) x (b, jh, f*4+sp)


            # ---- pre-warm: run the HAM clock-gate up to 8/8 during the
            # input-DMA head, and pre-load the Exp act table on ScalarE ----
            nc.scalar.activation(dummy_e[:, :], warm_t[:, 0:8], Exp)
            with tc.tile_pool(name="pwarm", bufs=1, space="PSUM") as pw_pool:
                pw = pw_pool.tile([A, 512], f32)
                for wi in range(10):
                    nc.tensor.matmul(
                        pw[:, :], lhsT=ones_bf[:, :], rhs=warm_t[:, :],
                        start=(wi == 0), stop=(wi == 9))

            # ---- Q/K projection + batched gathers ----
            def proj_stream(w_d, dest, dma_eng, extra=None):
                with tc.tile_pool(name="wtile", bufs=4) as w_pool, \
                     tc.tile_pool(name="stage", bufs=2) as st_pool, \
                     tc.tile_pool(name="pp", bufs=3, space="PSUM") as pp_pool:
                    stage = st_pool.tile([128, BC, NH, 128], f16,
                                         name="stage", tag="stage")
                    for tg in range(TTILES // WCHUNK):
                        wt = w_pool.tile([128, WCHUNK, KTILES, 128], f16,
                                         name="wt", tag="wt")
                        dma_eng.dma_start(
                            wt[:, :, :, :],
                            w_d[:, tg * WCHUNK:(tg + 1) * WCHUNK, :]
                            .rearrange("p t (kt c) -> p t kt c", c=128))
                        for gi in range(WCHUNK // GT):
                            pp = pp_pool.tile([128, GT, ROWS], f32)
                            for ti in range(GT):
                                for kt in range(KTILES):
                                    nc.tensor.matmul(
                                        pp[:, ti, :],
                                        lhsT=wt[:, gi * GT + ti, kt, :],
                                        rhs=hsT[:, kt, :],
                                        start=(kt == 0),
                                        stop=(kt == KTILES - 1))
                            # psum free (ti, b, nh, sp) -> iterate (bn, ti, sp)
                            src = pp.rearrange(
                                "p ti (b n sp) -> p (b n) ti sp", n=NH, sp=4)
                            t0 = tg * WCHUNK + gi * GT
                            dv = dest[:, 0, :, :, :].rearrange(
                                "p b n (f sp) -> p (b n) f sp", sp=4)
                            sv = stage[:, :, :, :].rearrange(
                                "p b n (f sp) -> p (b n) f sp", sp=4)
                            nc.vector.tensor_copy(
                                dv[:, :, t0:t0 + GT, :], src[0:64])
                            nc.vector.tensor_copy(
                                sv[64:128, :, t0:t0 + GT, :], src[64:128])
                        if extra is not None:
                            extra(tg)
                    # partition shift 64..127 -> 0..63 via SBUF->SBUF DMA on
                    # the scalar queue (64 contiguous 16KB runs; doesn't
                    # head-of-line-block the Wk prefetch on gpsimd).
                    nc.scalar.dma_start(
                        dest[:, 1, :, :, :],
                        stage[64:128, :, :, :])

            proj_stream(wq_d, qt, nc.sync)
            proj_stream(wk_d, kt_, nc.gpsimd)

            # ---- v projection (after Wk: its sparse PE window covers the
            # kt partition-shift DMA; warm-up below re-fires the clock) ----
            with tc.tile_pool(name="vps", bufs=4, space="PSUM") as vps_pool:
                for bpair in range(0, NB, 2):
                    vps = [vps_pool.tile([128, 2 * A], f32, name=f"vps{i}",
                                         tag=f"vps{i}")
                           for i in range(2)]
                    for pi in range(2):
                        nc.tensor.matmul(
                            vps[pi][:, :],
                            lhsT=hsv[pi * 64:(pi + 1) * 64, bpair // 2, :],
                            rhs=wv_sb[pi * 64:(pi + 1) * 64, :],
                            start=True, stop=True,
                            tile_position=(pi * 64, 0))
                    for pi in range(2):
                        nc.vector.tensor_copy(
                            v_all[:, bpair + pi, :, :], vps[pi][:, :])

            # ---- PE warm-up burst ----
            # Runs right after the Wk stream with no new dependencies, so it
            # executes DURING the kt-shift DMA tail: the PE stays busy (and
            # at full p-state) through the proj->attention transition.
            with tc.tile_pool(name="warm", bufs=1, space="PSUM") as wm_pool:
                wm = wm_pool.tile([A, ROWS], f32)
                for wi in range(24):
                    nc.tensor.matmul(
                        wm[:, :], lhsT=ones_bf[:, :], rhs=hsT[:, 0, :],
                        start=(wi == 0), stop=(wi == 23))

            # ---- attention (transpose-free, software-pipelined by 1) ----
            with tc.tile_pool(name="zps", bufs=4, space="PSUM") as z_pool, \
                 tc.tile_pool(name="dps", bufs=1, space="PSUM") as d_pool, \
                 tc.tile_pool(name="aps", bufs=1, space="PSUM") as a_pool, \
                 tc.tile_pool(name="expz", bufs=4) as e_pool, \
                 tc.tile_pool(name="reps", bufs=3) as rp_pool, \
                 tc.tile_pool(name="rps", bufs=2, space="PSUM") as r_pool, \
                 tc.tile_pool(name="fo", bufs=3) as f_pool:

                def emit_zt_exp(b):
                    ezs = []
                    for nh in range(NH):
                        zt = z_pool.tile([128, 2, 256], f32, name="zt",
                                         tag="zt")
                        for h in range(2):
                            nc.tensor.matmul(
                                zt[:, h, :],
                                lhsT=kt_[:, h, b, nh, :],
                                rhs=qt[:, :, b, nh, :],
                                start=True, stop=True)
                        ez = e_pool.tile([128, 2, 256], bf16, name="ez",
                                         tag="ez")
                        ezs.append(ez)
                        nc.scalar.activation(
                            ez[:, :, :].rearrange("p a b -> p (a b)"),
                            zt[:, :, :].rearrange("p a b -> p (a b)"), Exp)
                    return ezs

                def emit_resid(bp):
                    # residual for batch pair (2*bp, 2*bp+1): both output
                    # groups stacked on partition halves of one psum tile.
                    rp = r_pool.tile([128, 2, 128], f32, name="rp", tag="rp")
                    for g in range(2):
                        for jh in range(2):
                            nc.tensor.matmul(
                                rp[g * 64:(g + 1) * 64, :, :],
                                lhsT=wres_sb[g * 64:(g + 1) * 64, jh, :],
                                rhs=ut[g * 64:(g + 1) * 64,
                                       bp * 2:(bp + 1) * 2, jh, :],
                                start=(jh == 0), stop=(jh == 1),
                                tile_position=(g * 64, g * 64))
                    fo = f_pool.tile([128, 2, 128], f32, name="fo", tag="fo")
                    # fused bias + relu on VectorE (keeps ScalarE exp-only)
                    nc.vector.tensor_scalar(
                        fo[:, :, :].rearrange("p a b -> p (a b)"),
                        rp[:, :, :].rearrange("p a b -> p (a b)"),
                        bias_sb[:, :], 0.0, Add, Max)
                    nc.sync.dma_start(
                        out_d[:, bp * 256:(bp + 1) * 256],
                        fo[:, :, :].rearrange("p a b -> p (a b)"))

                ezs = emit_zt_exp(0)
                for b in range(BC):
                    cur = ezs
                    if b + 1 < BC:
                        ezs = emit_zt_exp(b + 1)
                    # denominators REPLICATED onto the right partition halves
                    # directly by the PE (ones lhsT, col-group packed per nh)
                    dpr = d_pool.tile([128, 256], f32)
                    for nh in range(NH):
                        for h in range(2):
                            nc.tensor.matmul(
                                dpr[nh * 64:(nh + 1) * 64, :],
                                lhsT=ones_bf[:, :],
                                rhs=cur[nh][:, h, :],
                                start=(h == 0), stop=(h == 1),
                                tile_position=(0, nh * 64))
                    rep = rp_pool.tile([128, 256], f32)
                    nc.vector.reciprocal_approx_fast(rep[:, :], dpr[:, :])
                    av = a_pool.tile([128, 256], f32)
                    for nh in range(NH):
                        bn = b * NH + nh
                        for kk in range(2):
                            nc.tensor.matmul(
                                av[nh * 64:(nh + 1) * 64, :],
                                lhsT=v_all[:, bn, kk, :],
                                rhs=cur[nh][:, kk, :],
                                start=(kk == 0), stop=(kk == 1),
                                tile_position=(0, nh * 64))
                    nc.vector.tensor_mul(
                        ut[:, b, :, :].rearrange("p a b -> p (a b)"),
                        av[:, :], rep[:, :])
                    # residual one pair behind, so the ut writes are done
                    if b >= 2 and b % 2 == 0:
                        emit_resid(b // 2 - 1)
                emit_resid(BC // 2 - 1)
    nc.compile()
    return nc


def _get_nc():
    global _NC_CACHE
    if _NC_CACHE is None:
        _NC_CACHE = build_bass()
    return _NC_CACHE


def _prep_weight(W):
    # (CD, ND) -> (128, TTILES, KTILES*128): [p, t, kt*128+j] = W[kt*128+p, t*128+j]
    return np.ascontiguousarray(
        W.astype(np.float16).reshape(KTILES, 128, TTILES, 128)
        .transpose(1, 2, 0, 3).reshape(128, TTILES, KTILES * 128))


def make_in_maps(Hs, Wq, Wk, Wv, Wres_w, Wres_b):
    wq16 = _prep_weight(Wq)
    wk16 = _prep_weight(Wk)
    wv16 = Wv.astype(np.float16)
    wres16 = Wres_w.astype(np.float16)
    bias = Wres_b.astype(np.float32).reshape(E, 1)
    hs16 = Hs.astype(np.float16)
    maps = []
    for c in range(NCORES):
        sh = hs16[c * BC:(c + 1) * BC]                      # (BC, S, CD)
        hs2d = sh.reshape(ROWS, CD)
        hst = np.ascontiguousarray(
            hs2d.reshape(ROWS, KTILES, 128).transpose(2, 1, 0))
        # v rows in sigma' order (f*4+sp):
        # hsv[pi*64+e, q, f*4+sp] = Hs[b, nh*4+sp, f, e]; bn = 2q+pi = b*NH+nh
        arr = sh.reshape(NB, 4, F, E).transpose(0, 2, 1, 3).reshape(NB, 128, E)
        hsv = np.ascontiguousarray(
            arr.reshape(NB // 2, 2, 128, E).transpose(1, 3, 0, 2)
            .reshape(128, NB // 2, 128))
        maps.append({
            "hst": hst, "hsv": hsv,
            "wq": wq16, "wk": wk16, "wv": wv16, "wres": wres16, "bias": bias,
        })
    return maps


def _unpack_out(o):
    # o: (128, (BC//2)*256) rows (g, e), cols (bp, b2, f, sp) -> (BC, S, F*E)
    o = o.reshape(2, 64, BC // 2, 2, F, 4)
    return np.ascontiguousarray(
        o.transpose(2, 3, 0, 5, 4, 1)).reshape(BC, S, F * E)


def kernel(Hs, Wq, Wk, Wv, Wres_w, Wres_b):
    from concourse.bass_utils import run_bass_kernel_spmd
    nc = _get_nc()
    in_maps = make_in_maps(Hs, Wq, Wk, Wv, Wres_w, Wres_b)
    res = run_bass_kernel_spmd(nc, in_maps, list(range(NCORES)))
    out = np.concatenate(
        [_unpack_out(np.asarray(res.results[c]["out"]))
         for c in range(NCORES)], axis=0)
    return out.astype(np.float32)


if __name__ == "__main__":
    nc = build_bass()
    print("built OK; instructions:",
          sum(len(bb.instructions) for fn in nc.m.functions
              for bb in fn.blocks))


# revision 11
# speedup vs baseline: 1.1558x; 1.0144x over previous
"""Trainium2 Bass kernel for nn_MultiHeadSelfAttention_88725434400988.

Self-contained: accepts FULL inputs, shards batch B=256 over 8 NeuronCores
(32 per core), runs one SPMD Bass program, gathers the FULL output.

Per-core algorithm (B_CORE=32, S=8, F=32, E=64, A=64, NH=2):
  - Hs, Wq, Wk, Wv, Wres cast to fp16 on host (PE matmuls run 1 cyc/row,
    fp32 PSUM accumulation; end-to-end error vs fp32 reference ~3.6e-3
    absmax / 1.9e-3 l2-relative).
  - HAM discipline: the PE clock-gate defaults to K=4/8 (1.2 GHz) and only
    reaches 2.4 GHz after ~3.4us of sustained busy; ANY ~3.4us idle window
    re-throttles. The whole kernel is structured to keep the PE dense:
    a pre-warm burst on memset data runs during the input-DMA head (so the
    Wq stream starts at full clock), the v projection is interleaved into
    the Wk stream (no sparse phase), and the attention loop is software-
    pipelined so it never stalls on ScalarE.
  - DMA queues: sync carries the Wq chunk stream + fo outputs ONLY (tiny
    wv/wres/bias DMAs go on gpsimd after hsv — each DIRECT2D issue costs
    ~650ns of engine time and was delaying the first Wq chunk); scalar
    (Activation HWDGE) carries hsT and the qt/kt partition-shift
    SBUF->SBUF DMAs so they never block the Wk prefetch on gpsimd.
  - QK projection: lhsT = 128-col tiles of W, rhs = Hs^T; psum groups of
    2 tiles (128=(jh,a), 2, 256=(b,s)); one batched copy per (group, jh).
    qt/kt are stored SIGMA-MAJOR ([64, jh, sigma, b, nh]) so the jh=1
    partition shift is a 64-descriptor contiguous-run DMA (~16KB/part);
    it is split in two halves issued as soon as their sigma range is
    complete (tg 7 / 15), hiding under the weight-stream tail.
  - Attention is TRANSPOSE-FREE: Z^T = k_chunk.T @ q (sigma on partition),
    exp on ScalarE -> bf16 (fp32 range, no overflow at |z|<=49). The act
    table is pre-loaded with a dummy Exp during the head. Denominators
    land REPLICATED on the correct partition halves straight from the PE
    (lhsT = ones(128,64), col-group packed per nh); reciprocal_approx_fast
    gives the recip tile and the normalization multiply is fused into the
    UT psum evacuation. Iteration b emits zt+exp for b+1 BEFORE den/av of
    b, so the ScalarE exp latency is covered by PE work.
  - Residual: both output groups stacked on partition halves of ONE psum
    tile (tile_position=(64,64) for g1), per 2-batch group, emitted one
    iteration late; bias+relu fused into one VectorE tensor_scalar (keeps
    ScalarE exp-only); contiguous DMA into a (128, 4096) staging output;
    host un-permutes to (B, S, F*E) fp32.
"""
import numpy as np

B, S, F, E, A, NH = 256, 8, 32, 64, 64, 2
NCORES = 8
BC = B // NCORES            # 32 batches per core
ROWS = BC * S               # 256 projection rows
CD = F * E                  # 2048 contraction dim
ND = A * F * NH             # 4096 projection cols
KTILES = CD // 128          # 16
TTILES = ND // 128          # 32 column tiles per weight
NB = BC * NH                # 64 attention batches per core
WCHUNK = 2                  # weight tiles per DMA
GT = 2                      # projection tiles batched per psum/copy group

_NC_CACHE = None


def build_bass():
    import concourse.bacc as bacc
    import concourse.tile as tile
    from concourse import mybir

    f16 = mybir.dt.float16
    bf16 = mybir.dt.bfloat16
    f32 = mybir.dt.float32
    Exp = mybir.ActivationFunctionType.Exp
    Copy = mybir.ActivationFunctionType.Copy
    Add = mybir.AluOpType.add
    Max = mybir.AluOpType.max

    nc = bacc.Bacc("TRN2", target_bir_lowering=False, debug=False)

    # host-prepped layouts (see make_in_maps)
    hst_d = nc.dram_tensor("hst", [128, KTILES, ROWS], f16, kind="ExternalInput")
    hsv_d = nc.dram_tensor("hsv", [128, NB // 2, 128], f16, kind="ExternalInput")
    wq_d = nc.dram_tensor("wq", [128, TTILES, KTILES * 128], f16,
                          kind="ExternalInput")
    wk_d = nc.dram_tensor("wk", [128, TTILES, KTILES * 128], f16,
                          kind="ExternalInput")
    wv_d = nc.dram_tensor("wv", [E, 2 * A], f16, kind="ExternalInput")
    wres_d = nc.dram_tensor("wres", [2 * A, E], f16, kind="ExternalInput")
    bias_d = nc.dram_tensor("bias", [E, 1], f32, kind="ExternalInput")
    out_d = nc.dram_tensor("out", [128, (BC // 2) * 256], f32,
                           kind="ExternalOutput")

    with tile.TileContext(nc) as tc:
        from contextlib import ExitStack
        with ExitStack() as ctx:
            singles = ctx.enter_context(tc.tile_pool(name="singles", bufs=1))

            # ---- constants / persistent tiles ----
            ones_bf = singles.tile([128, A], bf16)
            nc.vector.memset(ones_bf, 1.0)
            warm_t = singles.tile([128, 512], f16)
            nc.vector.memset(warm_t, 0.25)
            dummy_e = singles.tile([128, 8], bf16)

            hsT = singles.tile([128, KTILES, ROWS], f16)
            nc.scalar.dma_start(hsT[:, :, :], hst_d[:])
            hsv = singles.tile([128, NB // 2, 128], f16)

            wv_sb = singles.tile([128, 2 * A], f16)
            nc.gpsimd.dma_start(wv_sb[0:64, :], wv_d[:])
            nc.gpsimd.dma_start(wv_sb[64:128, :], wv_d[:])

            wres_sb = singles.tile([128, 2, E], f16)
            for half in range(2):
                for jh in range(2):
                    nc.gpsimd.dma_start(
                        wres_sb[half * 64:(half + 1) * 64, jh, :],
                        wres_d[jh * 64:(jh + 1) * 64, :])

            bias_sb = singles.tile([128, 1], f32)
            nc.gpsimd.dma_start(bias_sb[0:64, :], bias_d[:])
            nc.gpsimd.dma_start(bias_sb[64:128, :], bias_d[:])

            qt = singles.tile([64, 2, BC, NH, 128], f16)
            kt_ = singles.tile([64, 2, BC, NH, 128], f16)
            v_all = singles.tile([128, NB, 2, A], bf16)
            ut = singles.tile([128, BC, 2, 128], f16)  # (nh,a) x (b, jh, f*4+sp)


            # ---- pre-warm: run the HAM clock-gate up to 8/8 during the
            # input-DMA head, and pre-load the Exp act table on ScalarE ----
            nc.scalar.activation(dummy_e[:, :], warm_t[:, 0:8], Exp)
            with tc.tile_pool(name="pwarm", bufs=1, space="PSUM") as pw_pool:
                pw = pw_pool.tile([A, 512], f32)
                for wi in range(16):
                    nc.tensor.matmul(
                        pw[:, :], lhsT=ones_bf[:, :], rhs=warm_t[:, :],
                        start=(wi == 0), stop=(wi == 15))

            stage_q = singles.tile([128, BC, NH, 128], f16)
            stage_k = singles.tile([128, BC, NH, 128], f16)

            # ---- Q/K projection + batched gathers ----
            def proj_stream(w_d, dest, dma_eng, stage, extra=None):
                with tc.tile_pool(name="wtile", bufs=3) as w_pool, \
                     tc.tile_pool(name="pp", bufs=3, space="PSUM") as pp_pool:
                    for tg in range(TTILES // WCHUNK):
                        wt = w_pool.tile([128, WCHUNK, KTILES, 128], f16,
                                         name="wt", tag="wt")
                        dma_eng.dma_start(
                            wt[:, :, :, :],
                            w_d[:, tg * WCHUNK:(tg + 1) * WCHUNK, :]
                            .rearrange("p t (kt c) -> p t kt c", c=128))
                        for gi in range(WCHUNK // GT):
                            pp = pp_pool.tile([128, GT, ROWS], f32)
                            for ti in range(GT):
                                for kt in range(KTILES):
                                    nc.tensor.matmul(
                                        pp[:, ti, :],
                                        lhsT=wt[:, gi * GT + ti, kt, :],
                                        rhs=hsT[:, kt, :],
                                        start=(kt == 0),
                                        stop=(kt == KTILES - 1))
                            # psum free (ti, b, nh, sp) -> iterate (bn, ti, sp)
                            src = pp.rearrange(
                                "p ti (b n sp) -> p (b n) ti sp", n=NH, sp=4)
                            t0 = tg * WCHUNK + gi * GT
                            dv = dest[:, 0, :, :, :].rearrange(
                                "p b n (f sp) -> p (b n) f sp", sp=4)
                            sv = stage[:, :, :, :].rearrange(
                                "p b n (f sp) -> p (b n) f sp", sp=4)
                            nc.vector.tensor_copy(
                                dv[:, :, t0:t0 + GT, :], src[0:64])
                            nc.vector.tensor_copy(
                                sv[64:128, :, t0:t0 + GT, :], src[64:128])
                        if extra is not None:
                            extra(tg)
                    # partition shift 64..127 -> 0..63 via SBUF->SBUF DMA on
                    # the scalar queue (64 contiguous 16KB runs; doesn't
                    # head-of-line-block the Wk prefetch on gpsimd).
                    nc.scalar.dma_start(
                        dest[:, 1, :, :, :],
                        stage[64:128, :, :, :])

            proj_stream(wq_d, qt, nc.sync, stage_q)
            # hsv load sits on the scalar queue AFTER the qt shift: off the
            # bandwidth-critical head, in time for the v projection.
            nc.scalar.dma_start(hsv[:, :, :], hsv_d[:])
            proj_stream(wk_d, kt_, nc.gpsimd, stage_k)

            # ---- v projection (after Wk: its sparse PE window covers the
            # kt partition-shift DMA; warm-up below re-fires the clock) ----
            with tc.tile_pool(name="vps", bufs=4, space="PSUM") as vps_pool:
                for bpair in range(0, NB, 2):
                    vps = [vps_pool.tile([128, 2 * A], f32, name=f"vps{i}",
                                         tag=f"vps{i}")
                           for i in range(2)]
                    for pi in range(2):
                        nc.tensor.matmul(
                            vps[pi][:, :],
                            lhsT=hsv[pi * 64:(pi + 1) * 64, bpair // 2, :],
                            rhs=wv_sb[pi * 64:(pi + 1) * 64, :],
                            start=True, stop=True,
                            tile_position=(pi * 64, 0))
                    nc.scalar.activation(
                        v_all[:, bpair, :, :].rearrange("p a b -> p (a b)"),
                        vps[0][:, :], Copy)
                    nc.vector.tensor_copy(
                        v_all[:, bpair + 1, :, :], vps[1][:, :])

            # ---- PE warm-up burst ----
            # Runs right after the Wk stream with no new dependencies, so it
            # executes DURING the kt-shift DMA tail: the PE stays busy (and
            # at full p-state) through the proj->attention transition.
            with tc.tile_pool(name="warm", bufs=1, space="PSUM") as wm_pool:
                wm = wm_pool.tile([A, ROWS], f32)
                for wi in range(24):
                    nc.tensor.matmul(
                        wm[:, :], lhsT=ones_bf[:, :], rhs=hsT[:, 0, :],
                        start=(wi == 0), stop=(wi == 23))

            # ---- attention (transpose-free, software-pipelined by 1) ----
            with tc.tile_pool(name="zps", bufs=4, space="PSUM") as z_pool, \
                 tc.tile_pool(name="dps", bufs=1, space="PSUM") as d_pool, \
                 tc.tile_pool(name="aps", bufs=1, space="PSUM") as a_pool, \
                 tc.tile_pool(name="expz", bufs=4) as e_pool, \
                 tc.tile_pool(name="reps", bufs=3) as rp_pool, \
                 tc.tile_pool(name="rps", bufs=2, space="PSUM") as r_pool, \
                 tc.tile_pool(name="fo", bufs=3) as f_pool:

                def emit_zt_exp(b):
                    ezs = []
                    for nh in range(NH):
                        zt = z_pool.tile([128, 2, 256], f32, name="zt",
                                         tag="zt")
                        for h in range(2):
                            nc.tensor.matmul(
                                zt[:, h, :],
                                lhsT=kt_[:, h, b, nh, :],
                                rhs=qt[:, :, b, nh, :],
                                start=True, stop=True)
                        ez = e_pool.tile([128, 2, 256], bf16, name="ez",
                                         tag="ez")
                        ezs.append(ez)
                        nc.scalar.activation(
                            ez[:, :, :].rearrange("p a b -> p (a b)"),
                            zt[:, :, :].rearrange("p a b -> p (a b)"), Exp)
                    return ezs

                def emit_resid(bp):
                    # residual for batch pair (2*bp, 2*bp+1): both output
                    # groups stacked on partition halves of one psum tile.
                    rp = r_pool.tile([128, 2, 128], f32, name="rp", tag="rp")
                    for g in range(2):
                        for jh in range(2):
                            nc.tensor.matmul(
                                rp[g * 64:(g + 1) * 64, :, :],
                                lhsT=wres_sb[g * 64:(g + 1) * 64, jh, :],
                                rhs=ut[g * 64:(g + 1) * 64,
                                       bp * 2:(bp + 1) * 2, jh, :],
                                start=(jh == 0), stop=(jh == 1),
                                tile_position=(g * 64, g * 64))
                    fo = f_pool.tile([128, 2, 128], f32, name="fo", tag="fo")
                    # fused bias + relu on VectorE (keeps ScalarE exp-only)
                    nc.vector.tensor_scalar(
                        fo[:, :, :].rearrange("p a b -> p (a b)"),
                        rp[:, :, :].rearrange("p a b -> p (a b)"),
                        bias_sb[:, :], 0.0, Add, Max)
                    nc.sync.dma_start(
                        out_d[:, bp * 256:(bp + 1) * 256],
                        fo[:, :, :].rearrange("p a b -> p (a b)"))

                ezs = emit_zt_exp(0)
                for b in range(BC):
                    cur = ezs
                    if b + 1 < BC:
                        ezs = emit_zt_exp(b + 1)
                    # denominators REPLICATED onto the right partition halves
                    # directly by the PE (ones lhsT, col-group packed per nh)
                    dpr = d_pool.tile([128, 256], f32)
                    for nh in range(NH):
                        for h in range(2):
                            nc.tensor.matmul(
                                dpr[nh * 64:(nh + 1) * 64, :],
                                lhsT=ones_bf[:, :],
                                rhs=cur[nh][:, h, :],
                                start=(h == 0), stop=(h == 1),
                                tile_position=(0, nh * 64))
                    rep = rp_pool.tile([128, 256], f32)
                    nc.vector.reciprocal_approx_fast(rep[:, :], dpr[:, :])
                    av = a_pool.tile([128, 256], f32)
                    for nh in range(NH):
                        bn = b * NH + nh
                        for kk in range(2):
                            nc.tensor.matmul(
                                av[nh * 64:(nh + 1) * 64, :],
                                lhsT=v_all[:, bn, kk, :],
                                rhs=cur[nh][:, kk, :],
                                start=(kk == 0), stop=(kk == 1),
                                tile_position=(0, nh * 64))
                    nc.vector.tensor_mul(
                        ut[:, b, :, :].rearrange("p a b -> p (a b)"),
                        av[:, :], rep[:, :])
                    # residual one pair behind, so the ut writes are done
                    if b >= 2 and b % 2 == 0:
                        emit_resid(b // 2 - 1)
                emit_resid(BC // 2 - 1)
    nc.compile()
    return nc


def _get_nc():
    global _NC_CACHE
    if _NC_CACHE is None:
        _NC_CACHE = build_bass()
    return _NC_CACHE


def _prep_weight(W):
    # (CD, ND) -> (128, TTILES, KTILES*128): [p, t, kt*128+j] = W[kt*128+p, t*128+j]
    return np.ascontiguousarray(
        W.astype(np.float16).reshape(KTILES, 128, TTILES, 128)
        .transpose(1, 2, 0, 3).reshape(128, TTILES, KTILES * 128))


def make_in_maps(Hs, Wq, Wk, Wv, Wres_w, Wres_b):
    wq16 = _prep_weight(Wq)
    wk16 = _prep_weight(Wk)
    wv16 = Wv.astype(np.float16)
    wres16 = Wres_w.astype(np.float16)
    bias = Wres_b.astype(np.float32).reshape(E, 1)
    hs16 = Hs.astype(np.float16)
    maps = []
    for c in range(NCORES):
        sh = hs16[c * BC:(c + 1) * BC]                      # (BC, S, CD)
        hs2d = sh.reshape(ROWS, CD)
        hst = np.ascontiguousarray(
            hs2d.reshape(ROWS, KTILES, 128).transpose(2, 1, 0))
        # v rows in sigma' order (f*4+sp):
        # hsv[pi*64+e, q, f*4+sp] = Hs[b, nh*4+sp, f, e]; bn = 2q+pi = b*NH+nh
        arr = sh.reshape(NB, 4, F, E).transpose(0, 2, 1, 3).reshape(NB, 128, E)
        hsv = np.ascontiguousarray(
            arr.reshape(NB // 2, 2, 128, E).transpose(1, 3, 0, 2)
            .reshape(128, NB // 2, 128))
        maps.append({
            "hst": hst, "hsv": hsv,
            "wq": wq16, "wk": wk16, "wv": wv16, "wres": wres16, "bias": bias,
        })
    return maps


def _unpack_out(o):
    # o: (128, (BC//2)*256) rows (g, e), cols (bp, b2, f, sp) -> (BC, S, F*E)
    o = o.reshape(2, 64, BC // 2, 2, F, 4)
    return np.ascontiguousarray(
        o.transpose(2, 3, 0, 5, 4, 1)).reshape(BC, S, F * E)


def kernel(Hs, Wq, Wk, Wv, Wres_w, Wres_b):
    from concourse.bass_utils import run_bass_kernel_spmd
    nc = _get_nc()
    in_maps = make_in_maps(Hs, Wq, Wk, Wv, Wres_w, Wres_b)
    res = run_bass_kernel_spmd(nc, in_maps, list(range(NCORES)))
    out = np.concatenate(
        [_unpack_out(np.asarray(res.results[c]["out"]))
         for c in range(NCORES)], axis=0)
    return out.astype(np.float32)


if __name__ == "__main__":
    nc = build_bass()
    print("built OK; instructions:",
          sum(len(bb.instructions) for fn in nc.m.functions
              for bb in fn.blocks))
